# revision 70
# baseline (speedup 1.0000x reference)
"""Causal multi-head self-attention on 8 TRN2 NeuronCores (Bass/Tile).

Problem: z[B=2,T=2048,D=1024], per-head dim 64, H=16 heads, fp32.
Sharding: core = b*4 + g  (b = batch, g = head-group of 4 heads).

v2 pipeline (241.6us -> ~160us vs v1):
- Projection / outproj matmuls are software-pipelined into the
  attention chunks as rationed PE "filler" micro-units, so the PE
  never micro-idles waiting on exp (HAM throttled it to 1.2 GHz for
  ~93us of v1's span; now < ~15us).
- Q/K/V/P/z/w are bf16: same PE throughput, but LDWEIGHTS gets the
  2x fast-weight-load path, DMA halves, and DVE mask multiplies can
  take 2x modes.  Output partials are bf16 (summed fp32 on host).
  Measured rel err 4.4e-3 vs the 2e-2 gate.
- Denominators come out of the PV matmul itself: V tiles carry 64
  ones columns ([V|ones] even head slots, [ones|V] odd), so the ctx
  psum rows complementary to ctx hold the softmax denominator
  replicated 64x, partition-aligned for a fused evac-multiply.
  recip = exp(-ln(d)) on ScalarE (same ACT table set as the exps).
- Normalize of pair k is deferred past pair k+1's first scores;
  junk matmuls pinned to the tail keep the PE clock at 8/8 through
  the final normalize so the outproj drain runs at full rate.
"""
import sys
import types
from collections import deque

import ml_dtypes
import numpy as np

# ── antenv.axon_hooks shim (NTFF profiling; agent image lacks the module) ──
import antenv  # noqa: F401

if "antenv.axon_hooks" not in sys.modules:
    _hooks = types.ModuleType("antenv.axon_hooks")
    _HOOK = [None]
    _hooks.set_axon_ntff_profile_hook = lambda h: _HOOK.__setitem__(0, h)
    _hooks.get_axon_ntff_profile_hook = lambda: _HOOK[0]
    sys.modules["antenv.axon_hooks"] = _hooks
    antenv.axon_hooks = _hooks
    try:
        from trn_agent_boot.trn_boot import _ntff_profile_via_ctypes

        _hooks.set_axon_ntff_profile_hook(
            _ntff_profile_via_ctypes("/opt/axon/libaxon_pjrt.so")
        )
    except Exception:
        pass

import concourse.bass as bass  # noqa: E402
import concourse.tile as tile  # noqa: E402
import concourse.mybir as mybir  # noqa: E402
import concourse.bass_utils as bass_utils  # noqa: E402
from bass_rust import ScopedClock  # noqa: E402

bass_utils.upload_artifacts = lambda tmpdir: ""

F32 = mybir.dt.float32
F32R = mybir.dt.float32r
BF16 = mybir.dt.bfloat16
EXP = mybir.ActivationFunctionType.Exp
LN = mybir.ActivationFunctionType.Ln

# ── workaround: this walrus build allows max ONE sync-wait per instruction ──
_wsplit = [0]


def _split_excess_waits(nc, limit=1):
    n = 0
    for fn in nc.m.functions:
        for blk in fn.blocks:
            out = []
            for inst in blk.instructions:
                si = inst.sync_info
                if si is not None and len(si.on_wait) > limit:
                    ws = list(si.on_wait)
                    keep = ws[-limit:]
                    hoist = ws[:-limit]
                    for i in range(0, len(hoist), limit):
                        _wsplit[0] += 1
                        out.append(
                            mybir.InstNoOp(
                                name=f"I-wsplit-{_wsplit[0]}",
                                engine=inst.engine,
                                sync_info=mybir.SyncInfo(
                                    on_wait=hoist[i : i + limit], on_update=[]
                                ),
                                bass_nofuse=True,
                            )
                        )
                        n += 1
                    si.on_wait = keep
                out.append(inst)
            blk.instructions = out
    return n


def _patched_drain_and_barrier(self, tick_clock, wait_clock):
    nc = self.nc
    drain_inst = nc.sync.drain()
    wait_clock.add_sem_waits(
        drain_inst.ins, ScopedClock({None: tick_clock.global_clock})
    )
    si = drain_inst.ins.sync_info
    if si is not None and len(si.on_wait) > 1:
        waits = list(si.on_wait)
        si.on_wait = waits[:1]
        for w in waits[1:]:
            d2 = nc.sync.drain()
            d2.ins.sync_info = mybir.SyncInfo(on_wait=[w], on_update=[])
    nc.all_engine_barrier()
    assert self.sems is not None
    popped = nc._tile_sem_poison_stack.pop()
    assert popped is self._sem_poison
    nc.clear_and_free_semaphores(list(self.sems.allocated().values()))
    nc.all_engine_barrier()


tile.TileContext._drain_and_barrier = _patched_drain_and_barrier

# ── problem shape (hardcoded) ──
B, T, D, H, HD = 2, 2048, 1024, 16, 64
HPC = 4  # heads per core
DG = HPC * HD  # 256 projection cols per core
NQ = 512  # query-chunk width (one PSUM bank of fp32)
KT = T // 128  # 16 key tiles
NCH = T // NQ  # 4 query chunks
D8 = D // 128  # 8 contraction tiles
SCALE = 1.0 / np.sqrt(HD)


def build_kernel():
    nc = bass.Bass("TRN2", target_bir_lowering=False, debug=False)
    zt_d = nc.dram_tensor("zt", [D, T], BF16, kind="ExternalInput").ap()
    wq_d = nc.dram_tensor("wq", [D, DG], BF16, kind="ExternalInput").ap()
    wk_d = nc.dram_tensor("wk", [D, DG], BF16, kind="ExternalInput").ap()
    wv_d = nc.dram_tensor("wv", [D, DG], BF16, kind="ExternalInput").ap()
    wo_d = nc.dram_tensor("wo", [DG, D], BF16, kind="ExternalInput").ap()
    mk_d = nc.dram_tensor("mk", [128, 2 * 128], BF16, kind="ExternalInput").ap()
    on_d = nc.dram_tensor("on", [128, KT * HD], BF16, kind="ExternalInput").ap()
    ot_d = nc.dram_tensor("ot", [D, T], BF16, kind="ExternalOutput").ap()

    with tile.TileContext(nc) as tc:
        with tc.tile_pool(name="persist", bufs=1) as persist:
            wq_t = persist.tile([128, D8, DG], BF16)
            wk_t = persist.tile([128, D8, DG], BF16)
            wv_t = persist.tile([128, D8, DG], BF16)
            wo_t = persist.tile([128, DG // 128, D], BF16)
            mk_t = persist.tile([128, 2, 128], BF16)
            zt_t = persist.tile([128, D8, T], BF16)
            # head-pair stacked Q.T / K.T: partitions 0-63 head 2p, 64-127 head 2p+1
            # bf16: halves score-matmul LDWEIGHTS time via fast weight load
            qt_t = [persist.tile([128, T], BF16, tag=f"qt{p}", name=f"qt{p}") for p in range(2)]
            kt_t = [persist.tile([128, T], BF16, tag=f"kt{p}", name=f"kt{p}") for p in range(2)]
            # V per (key-tile, head-slot): even slots [V|ones], odd slots
            # [ones|V].  The ones half makes PV emit the softmax denominator
            # replicated on the complementary 64 psum rows, partition-aligned
            # with where the normalize multiply needs to write ct.
            v_t = persist.tile([128, KT, HPC, 128], BF16)
            # normalized ctx.T, stacked like qt (rows 0:63 head 2p, 64:127 head 2p+1)
            ct_t = [persist.tile([128, T], BF16, tag=f"ct{p}", name=f"ct{p}") for p in range(2)]
            # memset-initialized operand for HAM warmup matmuls: no DMA dep,
            # so the PE can start clocking immediately at kernel entry
            dum_t = persist.tile([128, DG], BF16)

            # DMA priority: wq + zt chunk 0 feed the first projections.
            nc.sync.dma_start(wq_t[:], wq_d.rearrange("(a p) c -> p a c", p=128))
            for k8 in range(D8):
                nc.sync.dma_start(
                    zt_t[:, k8, 0:NQ], zt_d[k8 * 128 : (k8 + 1) * 128, 0:NQ]
                )
            nc.sync.dma_start(wk_t[:], wk_d.rearrange("(a p) c -> p a c", p=128))
            nc.sync.dma_start(wv_t[:], wv_d.rearrange("(a p) c -> p a c", p=128))
            nc.sync.dma_start(mk_t[:], mk_d.rearrange("p (a b) -> p a b", a=2))
            # ones for key-tiles 0:4 only -- the rest queue after zt chunk 1,
            # which the chunk-0 filler (proj of chunk 1) stalls on otherwise
            on_r = on_d.rearrange("p (a b) -> p a b", a=KT)
            for h in range(HPC):
                cs = slice(HD, 128) if h % 2 == 0 else slice(0, HD)
                nc.sync.dma_start(v_t[:, 0:4, h, cs], on_r[:, 0:4])
            for k8 in range(D8):
                nc.sync.dma_start(
                    zt_t[:, k8, NQ : 2 * NQ],
                    zt_d[k8 * 128 : (k8 + 1) * 128, NQ : 2 * NQ],
                )
            for h in range(HPC):
                cs = slice(HD, 128) if h % 2 == 0 else slice(0, HD)
                nc.sync.dma_start(v_t[:, 4:KT, h, cs], on_r[:, 4:KT])
            for c in range(2, NCH):
                for k8 in range(D8):
                    nc.sync.dma_start(
                        zt_t[:, k8, c * NQ : (c + 1) * NQ],
                        zt_d[k8 * 128 : (k8 + 1) * 128, c * NQ : (c + 1) * NQ],
                    )
            nc.sync.dma_start(wo_t[:], wo_d.rearrange("(a p) c -> p a c", p=128))

            with (
                tc.tile_pool(name="pbuf", bufs=5) as pbuf,
                tc.tile_pool(name="nrm", bufs=2) as nrm,
                tc.tile_pool(name="stg", bufs=4) as stg,
                tc.tile_pool(name="ps_s", bufs=2, space="PSUM") as ps_s,
                tc.tile_pool(name="ps_c", bufs=2, space="PSUM") as ps_c,
                tc.tile_pool(name="ps_p", bufs=2, space="PSUM") as ps_p,
            ):
                # HAM warm-up on wq junk while zt chunk 0 streams in.
                nc.vector.memset(dum_t[:], 0)
                # trigger the exp/ln ACT table load during the DMA wait so
                # chunk 0's first softmax exp doesn't pay the ~2.7us switch
                # (output goes to a staging scratch so the warmup matmuls,
                # which read dum_t, don't wait on the scalar engine)
                atl = stg.tile([128, NQ], BF16, tag="st", name="atl")
                nc.scalar.activation(
                    out=atl[0:1, 0:64], in_=dum_t[0:1, 0:64], func=EXP
                )
                warm = ps_p.tile([128, NQ], F32, tag="pp", name="warm")
                for i in range(36):
                    nc.tensor.matmul(
                        warm[:, 0:DG],
                        dum_t[:, 0:128],
                        dum_t[:],
                        start=True,
                        stop=True,
                    )

                # Filler units are generators yielding after every ~2 matmuls
                # so pops can be rationed finely; PSUM accumulation groups in
                # different banks interleave freely on the PE.
                def gen_qk_unit(w_t, dst, m, c):
                    ps = ps_p.tile([128, NQ], F32, tag="pp", name="proj_ps")
                    for k8 in range(D8):
                        nc.tensor.matmul(
                            ps[:],
                            w_t[:, k8, m * 128 : (m + 1) * 128],
                            zt_t[:, k8, c * NQ : (c + 1) * NQ],
                            start=(k8 == 0),
                            stop=(k8 == D8 - 1),
                        )
                        if k8 % 2 == 1 and k8 < D8 - 1:
                            yield
                    nc.vector.tensor_copy(dst[m][:, c * NQ : (c + 1) * NQ], ps[:])

                def gen_v_unit(vm):
                    ps = ps_p.tile([128, NQ], F32, tag="pp", name="proj_ps")
                    for k8 in range(D8):
                        nc.tensor.matmul(
                            ps[:, 0:DG],
                            zt_t[:, k8, vm * 128 : (vm + 1) * 128],
                            wv_t[:, k8, :],
                            start=(k8 == 0),
                            stop=(k8 == D8 - 1),
                        )
                        if k8 % 4 == 3 and k8 < D8 - 1:
                            yield
                    # psum cols head-major (4x64); even head-slots keep V in
                    # cols 0:64, odd slots in cols 64:128 (ones elsewhere).
                    src = ps[:, 0:DG].rearrange("p (a c b) -> p a c b", a=2, c=2)
                    dst = v_t[:, vm].rearrange("p (a c) e -> p a c e", a=2)
                    nc.vector.tensor_copy(dst[:, :, 0, 0:HD], src[:, :, 0, :])
                    nc.vector.tensor_copy(dst[:, :, 1, HD:128], src[:, :, 1, :])

                def gen_out_unit(mo, c):
                    o_ps = ps_p.tile([128, NQ], F32, tag="pp", name="o_ps")
                    for kk in range(2):
                        nc.tensor.matmul(
                            o_ps[:],
                            wo_t[:, kk, mo * 128 : (mo + 1) * 128],
                            ct_t[kk][:, c * NQ : (c + 1) * NQ],
                            start=(kk == 0),
                            stop=(kk == 1),
                        )
                    yield
                    st = stg.tile([128, NQ], BF16, tag="st", name="st")
                    # only the final drain may borrow ScalarE for staging --
                    # mid-chunk it would delay the exp stream.
                    if c == NCH - 1 and mo % 2 == 0:
                        nc.scalar.copy(st[:], o_ps[:])
                    else:
                        nc.vector.tensor_copy(st[:], o_ps[:])
                    nc.sync.dma_start(
                        ot_d[mo * 128 : (mo + 1) * 128, c * NQ : (c + 1) * NQ],
                        st[:],
                    )

                def proj_gens(c):
                    gs = []
                    for w_t, dst in ((wq_t, qt_t), (wk_t, kt_t)):
                        for m in range(2):
                            gs.append(gen_qk_unit(w_t, dst, m, c))
                    for vm in range(4 * c, 4 * c + 4):
                        gs.append(gen_v_unit(vm))
                    return gs

                projq = deque()  # must drain before its chunk's attention
                outq = deque()  # deferred to fill the exp-heavy late chunks

                allow_out = [False]  # hold outproj filler for the late,
                # exp-heavy chunks where projection filler has run out

                def pop_filler(n):
                    for _ in range(n):
                        q = projq if projq else (outq if allow_out[0] else None)
                        if not q:
                            return
                        try:
                            next(q[0])
                        except StopIteration:
                            q.popleft()

                def drain(q):
                    while q:
                        try:
                            next(q[0])
                        except StopIteration:
                            q.popleft()

                # Q/K chunk-0 projections gate the first scores; the V
                # chunk-0 units overlap chunk-0's first exps, force-run
                # just before the PV that consumes each (emission order
                # defines dependency order, so they cannot be left to the
                # filler queue).
                g0 = proj_gens(0)
                projq.extend(g0[:4])
                drain(projq)
                v0_units = g0[4:]
                v0_state = [0]

                def ensure_v0(n):
                    while v0_state[0] < min(n, len(v0_units)):
                        for _ in v0_units[v0_state[0]]:
                            pass
                        v0_state[0] += 1

                pending_norm = [None]
                last_p = [None]

                for c in range(NCH):
                    if c + 1 < NCH:
                        projq.extend(proj_gens(c + 1))
                    nkt = 4 * c + 4
                    nb = nkt // 2
                    for p in range(2):
                        # hold deferred outproj filler for the exp-heavy
                        # chunk-3; chunk 2's first pair lives off projq alone
                        allow_out[0] = (c == 2 and p == 1) or c == 3
                        ctxs = [
                            ps_c.tile([128, NQ], F32, tag="ctx", name="ctx")
                            for _ in range(2)
                        ]
                        p_tiles = {}

                        def emit_scores_b(b, p=p, c=c, p_tiles=p_tiles):
                            # one [128, 2(head), 512] psum tile per key-tile;
                            # the two heads' matmuls go to PE row groups 0/64
                            # and run concurrently in the array.
                            for j in range(2):
                                kt = 2 * b + j
                                d = kt - 4 * c
                                lo = 128 * d if d > 0 else 0
                                s_ps = ps_s.tile(
                                    [128, 2, NQ], F32, tag="s", name="s_ps"
                                )
                                for h in range(2):
                                    hb = 64 * h
                                    nc.tensor.matmul(
                                        s_ps[:, h, lo:],
                                        kt_t[p][
                                            hb : hb + 64,
                                            kt * 128 : (kt + 1) * 128,
                                        ],
                                        qt_t[p][
                                            hb : hb + 64,
                                            c * NQ + lo : (c + 1) * NQ,
                                        ],
                                        start=True,
                                        stop=True,
                                    )
                                p_t = pbuf.tile(
                                    [128, 2, NQ], BF16, tag="p", name="p_t"
                                )
                                last_p[0] = p_t
                                nc.scalar.activation(
                                    out=p_t[:, :, lo:],
                                    in_=s_ps[:, :, lo:],
                                    func=EXP,
                                    scale=float(SCALE),
                                )
                                if d >= 0:
                                    band = p_t[:, :, lo : lo + 128]
                                    nc.vector.tensor_mul(band, band, mk_t[:])
                                p_tiles[kt] = (p_t, lo)

                        def emit_pv(b, h, p=p, c=c, p_tiles=p_tiles, ctxs=ctxs, nkt=nkt):
                            for j in range(2):
                                kt = 2 * b + j
                                p_t, lo = p_tiles[kt]
                                nc.tensor.matmul(
                                    ctxs[h][:, lo:],
                                    v_t[:, kt, 2 * p + h, :],
                                    p_t[:, h, lo:],
                                    start=(kt == 0),
                                    stop=(kt == nkt - 1),
                                )
                                if h == 1:
                                    p_tiles.pop(kt)

                        emit_scores_b(0)
                        # previous pair's normalize overlaps this pair's
                        # attention ramp-up instead of stalling it.
                        if pending_norm[0] is not None:
                            pending_norm[0]()
                            pending_norm[0] = None
                        pop_filler(1)
                        for b in range(nb):
                            if b + 1 < nb:
                                emit_scores_b(b + 1)
                            pop_filler(2)
                            if c == 0:
                                ensure_v0(2 * b + 2)
                            emit_pv(b, 0)
                            emit_pv(b, 1)
                            pop_filler(1)

                        # normalize: denominator sits replicated on the psum
                        # rows complementary to ctx (from the ones half of V).
                        def norm(c=c, p=p, ctxs=ctxs):
                            cw = slice(c * NQ, (c + 1) * NQ)
                            # recip = exp(-ln(d)) on ScalarE (same ACT table
                            # set as the softmax exps, so no table switch).
                            # The denominator slab sits on the psum rows
                            # complementary to ctx (ones half of V), already
                            # partition-aligned for the fused evac-multiply.
                            rc = nrm.tile([128, NQ], F32, tag="rc", name="rc")
                            nc.scalar.activation(
                                out=rc[64:128, :], in_=ctxs[0][64:128, :],
                                func=LN,
                            )
                            nc.scalar.activation(
                                out=rc[64:128, :], in_=rc[64:128, :], func=EXP,
                                scale=-1.0,
                            )
                            nc.vector.tensor_mul(
                                ct_t[p][0:64, cw],
                                ctxs[0][0:64, :],
                                rc[64:128, :],
                            )
                            nc.scalar.activation(
                                out=rc[0:64, :], in_=ctxs[1][0:64, :], func=LN
                            )
                            nc.scalar.activation(
                                out=rc[0:64, :], in_=rc[0:64, :], func=EXP,
                                scale=-1.0,
                            )
                            nc.vector.tensor_mul(
                                ct_t[p][64:128, cw],
                                ctxs[1][64:128, :],
                                rc[0:64, :],
                            )

                        pending_norm[0] = norm
                        pop_filler(2)
                    if c < NCH - 1:
                        for mo in range(D // 128):
                            outq.append(gen_out_unit(mo, c))
                    # next chunk's projections must be in SBUF before its
                    # attention reads them.
                    drain(projq)
                    if c == NCH - 1:
                        # keep the PE array clocked at 8/8 through the final
                        # normalize's serial scalar block so the outproj tail
                        # runs warm.  The junk matmuls read the last P tile so
                        # the scheduler cannot hoist them out of the tail.
                        warm2 = ps_p.tile([128, NQ], F32, tag="pp", name="warm")
                        for i in range(14):
                            nc.tensor.matmul(
                                warm2[:],
                                wq_t[:, i % D8, 0:128],
                                last_p[0][:, 1, :],
                                start=True,
                                stop=True,
                            )
                        pending_norm[0]()
                        pending_norm[0] = None
                        drain(outq)
                        # final outproj drain: scores psum banks are idle
                        # now, so spread the 8 accumulators over wide tiles to
                        # decouple the matmuls from the staging evacuations.
                        for mp in range(4):
                            os = ps_s.tile([128, 2, NQ], F32, tag="s", name="o2")
                            for j in range(2):
                                mo = 2 * mp + j
                                for kk in range(2):
                                    nc.tensor.matmul(
                                        os[:, j, :],
                                        wo_t[:, kk, mo * 128 : (mo + 1) * 128],
                                        ct_t[kk][:, c * NQ : (c + 1) * NQ],
                                        start=(kk == 0),
                                        stop=(kk == 1),
                                    )
                            for j in range(2):
                                mo = 2 * mp + j
                                st = stg.tile([128, NQ], BF16, tag="st", name="st")
                                if j == 0:
                                    nc.scalar.copy(st[:], os[:, j, :])
                                else:
                                    nc.vector.tensor_copy(st[:], os[:, j, :])
                                nc.sync.dma_start(
                                    ot_d[
                                        mo * 128 : (mo + 1) * 128,
                                        c * NQ : (c + 1) * NQ,
                                    ],
                                    st[:],
                                )

    return nc


def _host_inputs(z, w_q, w_k, w_v, w_o):
    """Per-core input maps (host-side sharding + transposes)."""
    z = np.asarray(z, dtype=np.float32)
    w_q = np.asarray(w_q, dtype=np.float32)
    w_k = np.asarray(w_k, dtype=np.float32)
    w_v = np.asarray(w_v, dtype=np.float32)
    w_o = np.asarray(w_o, dtype=np.float32)

    pj = np.arange(128)[:, None]
    fi = np.arange(128)[None, :]
    tri = (fi >= pj).astype(np.float32)  # [128, 128] causal boundary band
    mk2 = np.concatenate([tri, tri], axis=1)  # one band per head slot

    zt = [
        np.ascontiguousarray(z[b].T).astype(ml_dtypes.bfloat16) for b in range(B)
    ]
    in_maps = []
    for core in range(8):
        b, g = core // 4, core % 4
        cs = slice(g * DG, (g + 1) * DG)
        in_maps.append(
            {
                "zt": zt[b],
                "wq": np.ascontiguousarray(w_q[:, cs]).astype(ml_dtypes.bfloat16),
                "wk": np.ascontiguousarray(w_k[:, cs]).astype(ml_dtypes.bfloat16),
                "wv": np.ascontiguousarray(w_v[:, cs]).astype(ml_dtypes.bfloat16),
                "wo": np.ascontiguousarray(w_o[cs, :]).astype(ml_dtypes.bfloat16),
                "mk": mk2.astype(ml_dtypes.bfloat16),
                "on": np.ones((128, KT * HD), dtype=ml_dtypes.bfloat16),
            }
        )
    return in_maps


def run(z, w_q, w_k, w_v, w_o, trace=False, trace_cores=None):
    """Build + run on 8 cores; returns (output [B,T,D], BassKernelResults)."""
    nc = build_kernel()
    n = _split_excess_waits(nc)
    if n:
        print(f"[kernel] split {n} excess sync-waits onto nops", file=sys.stderr)
    in_maps = _host_inputs(z, w_q, w_k, w_v, w_o)
    res = bass_utils.run_bass_kernel_spmd(
        nc, in_maps, list(range(8)), trace=trace, trace_cores=trace_cores
    )
    out = np.zeros((B, T, D), dtype=np.float64)
    for core in range(8):
        out[core // 4] += res.results[core]["ot"].T.astype(np.float64)
    return out.astype(np.float32), res


def kernel(z, w_q, w_k, w_v, w_o):
    out, _ = run(z, w_q, w_k, w_v, w_o, trace=False)
    return out


# revision 71
# speedup vs baseline: 1.0093x; 1.0093x over previous
"""Causal multi-head self-attention on 8 TRN2 NeuronCores (Bass/Tile).

Problem: z[B=2,T=2048,D=1024], per-head dim 64, H=16 heads, fp32.
Sharding: core = b*4 + g  (b = batch, g = head-group of 4 heads).

v2 pipeline (241.6us -> ~160us vs v1):
- Projection / outproj matmuls are software-pipelined into the
  attention chunks as rationed PE "filler" micro-units, so the PE
  never micro-idles waiting on exp (HAM throttled it to 1.2 GHz for
  ~93us of v1's span; now < ~15us).
- Q/K/V/P/z/w are bf16: same PE throughput, but LDWEIGHTS gets the
  2x fast-weight-load path, DMA halves, and DVE mask multiplies can
  take 2x modes.  Output partials are bf16 (summed fp32 on host).
  Measured rel err 4.4e-3 vs the 2e-2 gate.
- Denominators come out of the PV matmul itself: V tiles carry 64
  ones columns ([V|ones] even head slots, [ones|V] odd), so the ctx
  psum rows complementary to ctx hold the softmax denominator
  replicated 64x, partition-aligned for a fused evac-multiply.
  recip = exp(-ln(d)) on ScalarE (same ACT table set as the exps).
- Normalize of pair k is deferred past pair k+1's first scores;
  junk matmuls pinned to the tail keep the PE clock at 8/8 through
  the final normalize so the outproj drain runs at full rate.
"""
import sys
import types
from collections import deque

import ml_dtypes
import numpy as np

# ── antenv.axon_hooks shim (NTFF profiling; agent image lacks the module) ──
import antenv  # noqa: F401

if "antenv.axon_hooks" not in sys.modules:
    _hooks = types.ModuleType("antenv.axon_hooks")
    _HOOK = [None]
    _hooks.set_axon_ntff_profile_hook = lambda h: _HOOK.__setitem__(0, h)
    _hooks.get_axon_ntff_profile_hook = lambda: _HOOK[0]
    sys.modules["antenv.axon_hooks"] = _hooks
    antenv.axon_hooks = _hooks
    try:
        from trn_agent_boot.trn_boot import _ntff_profile_via_ctypes

        _hooks.set_axon_ntff_profile_hook(
            _ntff_profile_via_ctypes("/opt/axon/libaxon_pjrt.so")
        )
    except Exception:
        pass

import concourse.bass as bass  # noqa: E402
import concourse.tile as tile  # noqa: E402
import concourse.mybir as mybir  # noqa: E402
import concourse.bass_utils as bass_utils  # noqa: E402
from bass_rust import ScopedClock  # noqa: E402

bass_utils.upload_artifacts = lambda tmpdir: ""

F32 = mybir.dt.float32
F32R = mybir.dt.float32r
BF16 = mybir.dt.bfloat16
EXP = mybir.ActivationFunctionType.Exp
LN = mybir.ActivationFunctionType.Ln

# ── workaround: this walrus build allows max ONE sync-wait per instruction ──
_wsplit = [0]


def _split_excess_waits(nc, limit=1):
    n = 0
    for fn in nc.m.functions:
        for blk in fn.blocks:
            out = []
            for inst in blk.instructions:
                si = inst.sync_info
                if si is not None and len(si.on_wait) > limit:
                    ws = list(si.on_wait)
                    keep = ws[-limit:]
                    hoist = ws[:-limit]
                    for i in range(0, len(hoist), limit):
                        _wsplit[0] += 1
                        out.append(
                            mybir.InstNoOp(
                                name=f"I-wsplit-{_wsplit[0]}",
                                engine=inst.engine,
                                sync_info=mybir.SyncInfo(
                                    on_wait=hoist[i : i + limit], on_update=[]
                                ),
                                bass_nofuse=True,
                            )
                        )
                        n += 1
                    si.on_wait = keep
                out.append(inst)
            blk.instructions = out
    return n


def _patched_drain_and_barrier(self, tick_clock, wait_clock):
    nc = self.nc
    drain_inst = nc.sync.drain()
    wait_clock.add_sem_waits(
        drain_inst.ins, ScopedClock({None: tick_clock.global_clock})
    )
    si = drain_inst.ins.sync_info
    if si is not None and len(si.on_wait) > 1:
        waits = list(si.on_wait)
        si.on_wait = waits[:1]
        for w in waits[1:]:
            d2 = nc.sync.drain()
            d2.ins.sync_info = mybir.SyncInfo(on_wait=[w], on_update=[])
    nc.all_engine_barrier()
    assert self.sems is not None
    popped = nc._tile_sem_poison_stack.pop()
    assert popped is self._sem_poison
    nc.clear_and_free_semaphores(list(self.sems.allocated().values()))
    nc.all_engine_barrier()


tile.TileContext._drain_and_barrier = _patched_drain_and_barrier

# ── problem shape (hardcoded) ──
B, T, D, H, HD = 2, 2048, 1024, 16, 64
HPC = 4  # heads per core
DG = HPC * HD  # 256 projection cols per core
NQ = 512  # query-chunk width (one PSUM bank of fp32)
KT = T // 128  # 16 key tiles
NCH = T // NQ  # 4 query chunks
D8 = D // 128  # 8 contraction tiles
SCALE = 1.0 / np.sqrt(HD)


def build_kernel():
    nc = bass.Bass("TRN2", target_bir_lowering=False, debug=False)
    zt_d = nc.dram_tensor("zt", [D, T], BF16, kind="ExternalInput").ap()
    wq_d = nc.dram_tensor("wq", [D, DG], BF16, kind="ExternalInput").ap()
    wk_d = nc.dram_tensor("wk", [D, DG], BF16, kind="ExternalInput").ap()
    wv_d = nc.dram_tensor("wv", [D, DG], BF16, kind="ExternalInput").ap()
    wo_d = nc.dram_tensor("wo", [DG, D], BF16, kind="ExternalInput").ap()
    mk_d = nc.dram_tensor("mk", [128, 2 * 128], BF16, kind="ExternalInput").ap()
    on_d = nc.dram_tensor("on", [128, KT * HD], BF16, kind="ExternalInput").ap()
    ot_d = nc.dram_tensor("ot", [D, T], BF16, kind="ExternalOutput").ap()

    with tile.TileContext(nc) as tc:
        with tc.tile_pool(name="persist", bufs=1) as persist:
            wq_t = persist.tile([128, D8, DG], BF16)
            wk_t = persist.tile([128, D8, DG], BF16)
            wv_t = persist.tile([128, D8, DG], BF16)
            wo_t = persist.tile([128, DG // 128, D], BF16)
            mk_t = persist.tile([128, 2, 128], BF16)
            zt_t = persist.tile([128, D8, T], BF16)
            # head-pair stacked Q.T / K.T: partitions 0-63 head 2p, 64-127 head 2p+1
            # bf16: halves score-matmul LDWEIGHTS time via fast weight load
            qt_t = [persist.tile([128, T], BF16, tag=f"qt{p}", name=f"qt{p}") for p in range(2)]
            kt_t = [persist.tile([128, T], BF16, tag=f"kt{p}", name=f"kt{p}") for p in range(2)]
            # V per (key-tile, head-slot): even slots [V|ones], odd slots
            # [ones|V].  The ones half makes PV emit the softmax denominator
            # replicated on the complementary 64 psum rows, partition-aligned
            # with where the normalize multiply needs to write ct.
            v_t = persist.tile([128, KT, HPC, 128], BF16)
            # normalized ctx.T, stacked like qt (rows 0:63 head 2p, 64:127 head 2p+1)
            ct_t = [persist.tile([128, T], BF16, tag=f"ct{p}", name=f"ct{p}") for p in range(2)]
            # memset-initialized operand for HAM warmup matmuls: no DMA dep,
            # so the PE can start clocking immediately at kernel entry
            dum_t = persist.tile([128, DG], BF16)

            # DMA priority: wq + zt chunk 0 feed the first projections.
            nc.sync.dma_start(wq_t[:], wq_d.rearrange("(a p) c -> p a c", p=128))
            for k8 in range(D8):
                nc.sync.dma_start(
                    zt_t[:, k8, 0:NQ], zt_d[k8 * 128 : (k8 + 1) * 128, 0:NQ]
                )
            nc.sync.dma_start(wk_t[:], wk_d.rearrange("(a p) c -> p a c", p=128))
            nc.sync.dma_start(wv_t[:], wv_d.rearrange("(a p) c -> p a c", p=128))
            nc.sync.dma_start(mk_t[:], mk_d.rearrange("p (a b) -> p a b", a=2))
            # ones for key-tiles 0:4 only -- the rest queue after zt chunk 1,
            # which the chunk-0 filler (proj of chunk 1) stalls on otherwise
            on_r = on_d.rearrange("p (a b) -> p a b", a=KT)
            for h in range(HPC):
                cs = slice(HD, 128) if h % 2 == 0 else slice(0, HD)
                nc.sync.dma_start(v_t[:, 0:4, h, cs], on_r[:, 0:4])
            for k8 in range(D8):
                nc.sync.dma_start(
                    zt_t[:, k8, NQ : 2 * NQ],
                    zt_d[k8 * 128 : (k8 + 1) * 128, NQ : 2 * NQ],
                )
            for h in range(HPC):
                cs = slice(HD, 128) if h % 2 == 0 else slice(0, HD)
                nc.sync.dma_start(v_t[:, 4:KT, h, cs], on_r[:, 4:KT])
            for c in range(2, NCH):
                for k8 in range(D8):
                    nc.sync.dma_start(
                        zt_t[:, k8, c * NQ : (c + 1) * NQ],
                        zt_d[k8 * 128 : (k8 + 1) * 128, c * NQ : (c + 1) * NQ],
                    )
            nc.sync.dma_start(wo_t[:], wo_d.rearrange("(a p) c -> p a c", p=128))

            with (
                tc.tile_pool(name="pbuf", bufs=5) as pbuf,
                tc.tile_pool(name="nrm", bufs=2) as nrm,
                tc.tile_pool(name="stg", bufs=4) as stg,
                tc.tile_pool(name="ps_s", bufs=2, space="PSUM") as ps_s,
                tc.tile_pool(name="ps_c", bufs=2, space="PSUM") as ps_c,
                tc.tile_pool(name="ps_p", bufs=2, space="PSUM") as ps_p,
            ):
                # HAM warm-up on wq junk while zt chunk 0 streams in.
                nc.vector.memset(dum_t[:], 0)
                # trigger the exp/ln ACT table load during the DMA wait so
                # chunk 0's first softmax exp doesn't pay the ~2.7us switch
                # (output goes to a staging scratch so the warmup matmuls,
                # which read dum_t, don't wait on the scalar engine)
                atl = stg.tile([128, NQ], BF16, tag="st", name="atl")
                nc.scalar.activation(
                    out=atl[0:1, 0:64], in_=dum_t[0:1, 0:64], func=EXP
                )
                warm = ps_p.tile([128, NQ], F32, tag="pp", name="warm")
                for i in range(36):
                    nc.tensor.matmul(
                        warm[:, 0:DG],
                        dum_t[:, 0:128],
                        dum_t[:],
                        start=True,
                        stop=True,
                    )

                # Filler units are generators yielding after every ~2 matmuls
                # so pops can be rationed finely; PSUM accumulation groups in
                # different banks interleave freely on the PE.
                def gen_qk_unit(w_t, dst, m, c):
                    ps = ps_p.tile([128, NQ], F32, tag="pp", name="proj_ps")
                    for k8 in range(D8):
                        nc.tensor.matmul(
                            ps[:],
                            w_t[:, k8, m * 128 : (m + 1) * 128],
                            zt_t[:, k8, c * NQ : (c + 1) * NQ],
                            start=(k8 == 0),
                            stop=(k8 == D8 - 1),
                        )
                        if k8 % 2 == 1 and k8 < D8 - 1:
                            yield
                    nc.vector.tensor_copy(dst[m][:, c * NQ : (c + 1) * NQ], ps[:])

                def gen_v_unit(vm):
                    ps = ps_p.tile([128, NQ], F32, tag="pp", name="proj_ps")
                    for k8 in range(D8):
                        nc.tensor.matmul(
                            ps[:, 0:DG],
                            zt_t[:, k8, vm * 128 : (vm + 1) * 128],
                            wv_t[:, k8, :],
                            start=(k8 == 0),
                            stop=(k8 == D8 - 1),
                        )
                        if k8 % 4 == 3 and k8 < D8 - 1:
                            yield
                    # psum cols head-major (4x64); even head-slots keep V in
                    # cols 0:64, odd slots in cols 64:128 (ones elsewhere).
                    src = ps[:, 0:DG].rearrange("p (a c b) -> p a c b", a=2, c=2)
                    dst = v_t[:, vm].rearrange("p (a c) e -> p a c e", a=2)
                    nc.vector.tensor_copy(dst[:, :, 0, 0:HD], src[:, :, 0, :])
                    nc.vector.tensor_copy(dst[:, :, 1, HD:128], src[:, :, 1, :])

                def gen_out_unit(mo, c):
                    o_ps = ps_p.tile([128, NQ], F32, tag="pp", name="o_ps")
                    for kk in range(2):
                        nc.tensor.matmul(
                            o_ps[:],
                            wo_t[:, kk, mo * 128 : (mo + 1) * 128],
                            ct_t[kk][:, c * NQ : (c + 1) * NQ],
                            start=(kk == 0),
                            stop=(kk == 1),
                        )
                    yield
                    st = stg.tile([128, NQ], BF16, tag="st", name="st")
                    # only the final drain may borrow ScalarE for staging --
                    # mid-chunk it would delay the exp stream.
                    if c == NCH - 1 and mo % 2 == 0:
                        nc.scalar.copy(st[:], o_ps[:])
                    else:
                        nc.vector.tensor_copy(st[:], o_ps[:])
                    nc.sync.dma_start(
                        ot_d[mo * 128 : (mo + 1) * 128, c * NQ : (c + 1) * NQ],
                        st[:],
                    )

                def proj_gens(c):
                    gs = []
                    for w_t, dst in ((wq_t, qt_t), (wk_t, kt_t)):
                        for m in range(2):
                            gs.append(gen_qk_unit(w_t, dst, m, c))
                    for vm in range(4 * c, 4 * c + 4):
                        gs.append(gen_v_unit(vm))
                    return gs

                projq = deque()  # must drain before its chunk's attention
                outq = deque()  # deferred to fill the exp-heavy late chunks

                allow_out = [False]  # hold outproj filler for the late,
                # exp-heavy chunks where projection filler has run out

                def pop_filler(n):
                    for _ in range(n):
                        q = projq if projq else (outq if allow_out[0] else None)
                        if not q:
                            return
                        try:
                            next(q[0])
                        except StopIteration:
                            q.popleft()

                def drain(q):
                    while q:
                        try:
                            next(q[0])
                        except StopIteration:
                            q.popleft()

                # Q/K chunk-0 projections gate the first scores; the V
                # chunk-0 units overlap chunk-0's first exps, force-run
                # just before the PV that consumes each (emission order
                # defines dependency order, so they cannot be left to the
                # filler queue).
                g0 = proj_gens(0)
                projq.extend(g0[:4])
                drain(projq)
                v0_units = g0[4:]
                v0_state = [0]

                def ensure_v0(n):
                    while v0_state[0] < min(n, len(v0_units)):
                        for _ in v0_units[v0_state[0]]:
                            pass
                        v0_state[0] += 1

                pending_norm = [None]
                last_p = [None]

                for c in range(NCH):
                    if c + 1 < NCH:
                        projq.extend(proj_gens(c + 1))
                    allow_out[0] = c >= 2
                    nkt = 4 * c + 4
                    nb = nkt // 2
                    for p in range(2):
                        ctxs = [
                            ps_c.tile([128, NQ], F32, tag="ctx", name="ctx")
                            for _ in range(2)
                        ]
                        p_tiles = {}

                        def emit_scores_b(b, p=p, c=c, p_tiles=p_tiles):
                            # one [128, 2(head), 512] psum tile per key-tile;
                            # the two heads' matmuls go to PE row groups 0/64
                            # and run concurrently in the array.
                            for j in range(2):
                                kt = 2 * b + j
                                d = kt - 4 * c
                                lo = 128 * d if d > 0 else 0
                                s_ps = ps_s.tile(
                                    [128, 2, NQ], F32, tag="s", name="s_ps"
                                )
                                for h in range(2):
                                    hb = 64 * h
                                    nc.tensor.matmul(
                                        s_ps[:, h, lo:],
                                        kt_t[p][
                                            hb : hb + 64,
                                            kt * 128 : (kt + 1) * 128,
                                        ],
                                        qt_t[p][
                                            hb : hb + 64,
                                            c * NQ + lo : (c + 1) * NQ,
                                        ],
                                        start=True,
                                        stop=True,
                                    )
                                p_t = pbuf.tile(
                                    [128, 2, NQ], BF16, tag="p", name="p_t"
                                )
                                last_p[0] = p_t
                                nc.scalar.activation(
                                    out=p_t[:, :, lo:],
                                    in_=s_ps[:, :, lo:],
                                    func=EXP,
                                    scale=float(SCALE),
                                )
                                if d >= 0:
                                    band = p_t[:, :, lo : lo + 128]
                                    nc.vector.tensor_mul(band, band, mk_t[:])
                                p_tiles[kt] = (p_t, lo)

                        def emit_pv(b, h, p=p, c=c, p_tiles=p_tiles, ctxs=ctxs, nkt=nkt):
                            for j in range(2):
                                kt = 2 * b + j
                                p_t, lo = p_tiles[kt]
                                nc.tensor.matmul(
                                    ctxs[h][:, lo:],
                                    v_t[:, kt, 2 * p + h, :],
                                    p_t[:, h, lo:],
                                    start=(kt == 0),
                                    stop=(kt == nkt - 1),
                                )
                                if h == 1:
                                    p_tiles.pop(kt)

                        emit_scores_b(0)
                        # previous pair's normalize overlaps this pair's
                        # attention ramp-up instead of stalling it.
                        if pending_norm[0] is not None:
                            pending_norm[0]()
                            pending_norm[0] = None
                        pop_filler(1)
                        for b in range(nb):
                            if b + 1 < nb:
                                emit_scores_b(b + 1)
                            pop_filler(2)
                            if c == 0:
                                ensure_v0(2 * b + 2)
                            emit_pv(b, 0)
                            emit_pv(b, 1)
                            pop_filler(1)

                        # normalize: denominator sits replicated on the psum
                        # rows complementary to ctx (from the ones half of V).
                        def norm(c=c, p=p, ctxs=ctxs):
                            cw = slice(c * NQ, (c + 1) * NQ)
                            # recip = exp(-ln(d)) on ScalarE (same ACT table
                            # set as the softmax exps, so no table switch).
                            # The denominator slab sits on the psum rows
                            # complementary to ctx (ones half of V), already
                            # partition-aligned for the fused evac-multiply.
                            rc = nrm.tile([128, NQ], F32, tag="rc", name="rc")
                            nc.scalar.activation(
                                out=rc[64:128, :], in_=ctxs[0][64:128, :],
                                func=LN,
                            )
                            nc.scalar.activation(
                                out=rc[64:128, :], in_=rc[64:128, :], func=EXP,
                                scale=-1.0,
                            )
                            nc.vector.tensor_mul(
                                ct_t[p][0:64, cw],
                                ctxs[0][0:64, :],
                                rc[64:128, :],
                            )
                            nc.scalar.activation(
                                out=rc[0:64, :], in_=ctxs[1][0:64, :], func=LN
                            )
                            nc.scalar.activation(
                                out=rc[0:64, :], in_=rc[0:64, :], func=EXP,
                                scale=-1.0,
                            )
                            nc.vector.tensor_mul(
                                ct_t[p][64:128, cw],
                                ctxs[1][64:128, :],
                                rc[0:64, :],
                            )

                        pending_norm[0] = norm
                        pop_filler(2)
                    if c < NCH - 1:
                        for mo in range(D // 128):
                            outq.append(gen_out_unit(mo, c))
                    # next chunk's projections must be in SBUF before its
                    # attention reads them.
                    drain(projq)
                    if c == NCH - 1:
                        # keep the PE array clocked at 8/8 through the final
                        # normalize's serial scalar block so the outproj tail
                        # runs warm.  The junk matmuls read the last P tile so
                        # the scheduler cannot hoist them out of the tail.
                        warm2 = ps_p.tile([128, NQ], F32, tag="pp", name="warm")
                        for i in range(14):
                            nc.tensor.matmul(
                                warm2[:],
                                wq_t[:, i % D8, 0:128],
                                last_p[0][:, 1, :],
                                start=True,
                                stop=True,
                            )
                        pending_norm[0]()
                        pending_norm[0] = None
                        drain(outq)
                        # final outproj drain: scores psum banks are idle
                        # now, so spread the 8 accumulators over wide tiles to
                        # decouple the matmuls from the staging evacuations.
                        for mp in range(4):
                            os = ps_s.tile([128, 2, NQ], F32, tag="s", name="o2")
                            for j in range(2):
                                mo = 2 * mp + j
                                for kk in range(2):
                                    nc.tensor.matmul(
                                        os[:, j, :],
                                        wo_t[:, kk, mo * 128 : (mo + 1) * 128],
                                        ct_t[kk][:, c * NQ : (c + 1) * NQ],
                                        start=(kk == 0),
                                        stop=(kk == 1),
                                    )
                            for j in range(2):
                                mo = 2 * mp + j
                                st = stg.tile([128, NQ], BF16, tag="st", name="st")
                                if j == 0:
                                    nc.scalar.copy(st[:], os[:, j, :])
                                else:
                                    nc.vector.tensor_copy(st[:], os[:, j, :])
                                nc.sync.dma_start(
                                    ot_d[
                                        mo * 128 : (mo + 1) * 128,
                                        c * NQ : (c + 1) * NQ,
                                    ],
                                    st[:],
                                )

    return nc


def _host_inputs(z, w_q, w_k, w_v, w_o):
    """Per-core input maps (host-side sharding + transposes)."""
    z = np.asarray(z, dtype=np.float32)
    w_q = np.asarray(w_q, dtype=np.float32)
    w_k = np.asarray(w_k, dtype=np.float32)
    w_v = np.asarray(w_v, dtype=np.float32)
    w_o = np.asarray(w_o, dtype=np.float32)

    pj = np.arange(128)[:, None]
    fi = np.arange(128)[None, :]
    tri = (fi >= pj).astype(np.float32)  # [128, 128] causal boundary band
    mk2 = np.concatenate([tri, tri], axis=1)  # one band per head slot

    zt = [
        np.ascontiguousarray(z[b].T).astype(ml_dtypes.bfloat16) for b in range(B)
    ]
    in_maps = []
    for core in range(8):
        b, g = core // 4, core % 4
        cs = slice(g * DG, (g + 1) * DG)
        in_maps.append(
            {
                "zt": zt[b],
                "wq": np.ascontiguousarray(w_q[:, cs]).astype(ml_dtypes.bfloat16),
                "wk": np.ascontiguousarray(w_k[:, cs]).astype(ml_dtypes.bfloat16),
                "wv": np.ascontiguousarray(w_v[:, cs]).astype(ml_dtypes.bfloat16),
                "wo": np.ascontiguousarray(w_o[cs, :]).astype(ml_dtypes.bfloat16),
                "mk": mk2.astype(ml_dtypes.bfloat16),
                "on": np.ones((128, KT * HD), dtype=ml_dtypes.bfloat16),
            }
        )
    return in_maps


def run(z, w_q, w_k, w_v, w_o, trace=False, trace_cores=None):
    """Build + run on 8 cores; returns (output [B,T,D], BassKernelResults)."""
    nc = build_kernel()
    n = _split_excess_waits(nc)
    if n:
        print(f"[kernel] split {n} excess sync-waits onto nops", file=sys.stderr)
    in_maps = _host_inputs(z, w_q, w_k, w_v, w_o)
    res = bass_utils.run_bass_kernel_spmd(
        nc, in_maps, list(range(8)), trace=trace, trace_cores=trace_cores
    )
    out = np.zeros((B, T, D), dtype=np.float64)
    for core in range(8):
        out[core // 4] += res.results[core]["ot"].T.astype(np.float64)
    return out.astype(np.float32), res


def kernel(z, w_q, w_k, w_v, w_o):
    out, _ = run(z, w_q, w_k, w_v, w_o, trace=False)
    return out
